# revision 41
# baseline (speedup 1.0000x reference)
"""GPT-NeoX attention (B=4, S=1024, D=2048, H=16) on 8 TRN2 NeuronCores.

Tensor-parallel over heads: 2 heads per core. Each core computes its slice
of the fused QKV projection, RoPE, causal attention, and writes the
transposed per-head output [hd, S]; the host concatenates heads.

All on-chip matmuls use float32r (full PE speed at free-dim>=256) with
fp32 PSUM accumulation. Layouts avoid on-chip transposes:
  - x is fed transposed  xT[feature, token]
  - q,k are produced transposed  qT/kT[hd, token]  (RoPE applied in place)
  - v is produced natural  v[token, hd]  via x-stationary matmuls
  - scores are computed transposed  sT[k_token, q_token]
  - out is produced transposed  oT[hd, q_token] = v.T @ expT
  - softmax sum over k = ones-matmul; normalization applied to oT

Scheduling structure (vs the naive version):
  - x and w stream per-kc chunk on separate DMA queues so the first
    matmul waits for ~320KB, not the whole 10.5MB prefetch.
  - The projection loop is kc-outer with 4 open PSUM groups, so compute
    tracks chunk arrival during the DMA-paced first batch.
  - Attention runs on 256-wide q-chunks (finer causal trim) and is
    software-pipelined into the RoPE/bias windows of the next half.
"""

import os

import numpy as np

import ml_dtypes

import concourse.bass as bass
import concourse.tile as tile
from concourse import bacc, mybir

# Problem constants (contract: nn_GPTNeoXAttention, fixed shapes)
B, S, D = 4, 1024, 2048
H = 16
HD = 128  # head dim
NCORES = 8
HPC = H // NCORES  # heads per core
ROPE_BASE = 10000.0
T = B * S  # 4096 tokens
KC = D // 128  # 16 contraction chunks of the model dim
NSL = 512  # token-slice width (half a sequence)
QW = 256  # attention q-chunk width
SCALE = 1.0 / float(np.sqrt(HD))

F32 = mybir.dt.float32
F32R = mybir.dt.float32r
BF16 = mybir.dt.bfloat16

_CACHE = {}


def _build_program():
    nc = bacc.Bacc(
        "TRN2", target_bir_lowering=False, debug=False, num_devices=NCORES
    )

    x_d = nc.dram_tensor("x", [128, KC, T], BF16, kind="ExternalInput")
    w_d = nc.dram_tensor("w", [128, KC, 6 * HD], BF16, kind="ExternalInput")
    bqk_d = nc.dram_tensor("bqk", [128, 4], F32, kind="ExternalInput")
    bv_d = nc.dram_tensor("bv", [128, 2 * HD], F32, kind="ExternalInput")
    cos_d = nc.dram_tensor("cosT", [128, S], BF16, kind="ExternalInput")
    sin_d = nc.dram_tensor("sinS", [128, S], BF16, kind="ExternalInput")
    mask_d = nc.dram_tensor("masks", [128, 2, QW], BF16, kind="ExternalInput")
    ones_d = nc.dram_tensor("ones", [128, 128], F32R, kind="ExternalInput")
    out_d = nc.dram_tensor("out", [HPC, HD, B, S], BF16, kind="ExternalOutput")

    x_ap = x_d.ap()
    w_ap = w_d.ap()
    out_ap = out_d.ap()

    Exp = mybir.ActivationFunctionType.Exp
    Identity = mybir.ActivationFunctionType.Identity

    with tile.TileContext(nc) as tc:
        with (
            tc.tile_pool(name="singles", bufs=1) as singles,
            tc.tile_pool(name="xin", bufs=2) as xin_pool,
            tc.tile_pool(name="qk", bufs=8) as qk_pool,
            tc.tile_pool(name="vp", bufs=2) as v_pool,
            tc.tile_pool(name="qbp", bufs=3) as qb_pool,
            tc.tile_pool(name="t2p", bufs=2) as t2_pool,
            tc.tile_pool(name="expp", bufs=8) as exp_pool,
            tc.tile_pool(name="outp", bufs=4) as out_pool,
            tc.tile_pool(name="rcp", bufs=3) as rcp_pool,
            tc.tile_pool(name="accp", bufs=3) as acc_pool,
            tc.tile_pool(name="ps_qk", bufs=2, space="PSUM") as ps_qk,
            tc.tile_pool(name="ps_v", bufs=2, space="PSUM") as ps_v,
            tc.tile_pool(name="ps_s", bufs=2, space="PSUM") as ps_s,
            tc.tile_pool(name="ps_o", bufs=2, space="PSUM") as ps_o,
        ):
            # --- HAM warmup: dummy matmuls run during the DMA-bound
            # prefetch so the PE clock-gate opens before real compute ---
            scratch = singles.tile([128, NSL], F32, tag="scratch")
            nc.gpsimd.memset(scratch[:], 0.0)
            scratch_bf = singles.tile([128, NSL], BF16, tag="scratchb")
            nc.gpsimd.memset(scratch_bf[:], 0.0)
            for wi in range(6):
                pw = ps_qk.tile([128, NSL], F32, tag="qk", name=f"warm_{wi}")
                nc.tensor.matmul(
                    pw, scratch[:, :128], scratch, start=True, stop=True
                )

            # --- weights stream per-kc, split by use: the q-head columns
            # arrive first so the first projection sweep is paced by ~1MB,
            # then k columns; v columns ride the sync queue after singles ---
            w_sb = []
            for kc in range(KC):
                wt = singles.tile(
                    [128, 6 * HD], BF16, tag=f"w{kc}", name=f"w_sb_{kc}"
                )
                w_sb.append(wt)
            for kc in range(KC):
                nc.scalar.dma_start(
                    out=w_sb[kc][:, 0:256], in_=w_ap[:, kc, 0:256]
                )
            for kc in range(KC):
                nc.scalar.dma_start(
                    out=w_sb[kc][:, 256:512], in_=w_ap[:, kc, 256:512]
                )
            # --- sync queue, ordered by first use: tiny constants and
            # the v-projection weight columns first, then the RoPE tables ---
            bqk_sb = singles.tile([128, 4], F32, tag="bqk")
            nc.sync.dma_start(out=bqk_sb, in_=bqk_d.ap())
            mask_sb = singles.tile([128, 2, QW], BF16, tag="mask")
            nc.sync.dma_start(out=mask_sb, in_=mask_d.ap())
            # ones[128,128] lhsT: ones.T @ expT = sum over k on all partitions
            ones_sb = singles.tile([128, 128], F32R, tag="ones")
            nc.sync.dma_start(out=ones_sb, in_=ones_d.ap())
            for kc in range(KC):
                nc.sync.dma_start(
                    out=w_sb[kc][:, 512:768], in_=w_ap[:, kc, 512:768]
                )
            cos_sb = singles.tile([128, S], BF16, tag="cos")
            nc.sync.dma_start(out=cos_sb, in_=cos_d.ap())
            sin_sb = singles.tile([128, S], BF16, tag="sin")
            nc.sync.dma_start(out=sin_sb, in_=sin_d.ap())
            bv_sb = singles.tile([128, 2 * HD], F32, tag="bv")
            nc.sync.dma_start(out=bv_sb, in_=bv_d.ap())

            qk_tiles = {}  # (b, m) -> feature-major q/k tile [128, S]
            v_tiles = {}  # b -> natural v tile [128, 8, 2*HD]

            def issue_x(b, half, eng=None):
                t0 = b * S + half * NSL
                xs = []
                for kc in range(KC):
                    xt = xin_pool.tile(
                        [128, NSL], BF16, tag=f"x{kc}", name=f"x_{b}_{half}_{kc}"
                    )
                    (eng or nc.gpsimd).dma_start(
                        out=xt, in_=x_ap[:, kc, t0 : t0 + NSL]
                    )
                    xs.append(xt)
                return xs

            def proj_sweep(b, half, xs, sweep):
                """q heads (sweep=0) or k heads (sweep=1): two qk PSUM
                groups accumulated kc-outer so compute tracks DMA arrival."""
                pqs = [
                    ps_qk.tile(
                        [128, NSL], F32, tag="qk", name=f"pq_{b}_{half}_{sweep}_{i}"
                    )
                    for i in range(2)
                ]
                for kc in range(KC):
                    for i in range(2):
                        m = 2 * sweep + i
                        nc.tensor.matmul(
                            pqs[i],
                            w_sb[kc][:, m * 128 : (m + 1) * 128],
                            xs[kc],
                            start=(kc == 0),
                            stop=(kc == KC - 1),
                        )
                    nfill = 1 if (b == 0 and half == 0 and kc < KC - 1) else 0
                    for fi in range(nfill):
                        # keep the PE clock-gate open while DMA paces b0:
                        # dependency-free filler matmuls bridge the gap
                        pf = ps_s.tile(
                            [128, 2, QW], F32, tag="sc",
                            name=f"fill_{b}_{half}_{sweep}_{kc}_{fi}",
                        )
                        nc.tensor.matmul(
                            pf[:, :, :], scratch_bf[:, :128], scratch_bf,
                            start=True, stop=True,
                        )
                return pqs

            def v_pass(b, half, xs, pair):
                """natural-layout v for token tiles (2*pair, 2*pair+1)."""
                for i in range(2):
                    t = 2 * pair + i
                    pv = ps_v.tile(
                        [128, 2 * HD], F32, tag="v", name=f"pv_{b}_{half}_{t}"
                    )
                    for kc in range(KC):
                        nc.tensor.matmul(
                            pv,
                            xs[kc][:, t * 128 : (t + 1) * 128],
                            w_sb[kc][:, 4 * 128 : 6 * 128],
                            start=(kc == 0),
                            stop=(kc == KC - 1),
                        )
                    nc.vector.tensor_add(
                        v_tiles[b][:, half * 4 + t, :], pv, bv_sb
                    )

            def rope_pair(b, half, sweep, pqs):
                """bias + RoPE for the two feature blocks of one sweep.
                rotate_half = partition rotation by 64 via SBUF-to-SBUF DMA
                (sign folded into the sin table)."""
                sl = slice(half * NSL, (half + 1) * NSL)
                for i in range(2):
                    m = 2 * sweep + i
                    qb = qb_pool.tile(
                        [128, NSL], F32R, tag="qb", name=f"qb_{b}_{half}_{m}"
                    )
                    nc.scalar.activation(
                        qb, pqs[i], Identity, bias=bqk_sb[:, m : m + 1], scale=1.0
                    )
                    qrot = t2_pool.tile(
                        [128, NSL], F32R, tag="t2", name=f"t2_{b}_{half}_{m}"
                    )
                    nc.sync.dma_start(out=qrot[0:64, :], in_=qb[64:128, :])
                    nc.sync.dma_start(out=qrot[64:128, :], in_=qb[0:64, :])
                    dst = qk_tiles[(b, m)][:, sl]
                    nc.vector.tensor_mul(qrot, qrot, sin_sb[:, sl])
                    nc.vector.tensor_mul(dst, qb, cos_sb[:, sl])
                    nc.vector.tensor_add(dst, dst, qrot)

            def attn_scores(b, h, qs, fill=False, vjobs=None, veng=None):
                """scores/exp/mask/AV for one 256-wide q-chunk. Score chunks
                are paired into one PSUM bank (one accumulation group writing
                disjoint halves) so exp and the DVE denominator accumulation
                run on [128,512] tiles. fill=True inserts dependency-free
                filler matmuls so the PE clock-gate stays open while the
                chunk chain waits on ACT/DVE."""
                nk = 2 * (qs + 1)
                qT = qk_tiles[(b, h)]
                kT = qk_tiles[(b, 2 + h)]
                qsl = slice(qs * QW, (qs + 1) * QW)
                ps_out = ps_o.tile([128, QW], F32, tag="po", name=f"po_{b}_{h}_{qs}")
                acc = acc_pool.tile(
                    [128, 2, QW], F32R, tag="acc", name=f"acc_{b}_{h}_{qs}"
                )
                for kp in range(nk // 2):
                    pss = ps_s.tile(
                        [128, 2, QW], F32, tag="sc", name=f"sc_{b}_{h}_{qs}_{kp}"
                    )
                    for j in range(2):
                        ki = 2 * kp + j
                        nc.tensor.matmul(
                            pss[:, j, :],
                            kT[:, ki * 128 : (ki + 1) * 128],
                            qT[:, qsl],
                            start=(j == 0),
                            stop=(j == 1),
                        )
                    e = exp_pool.tile(
                        [128, 2, QW], F32R, tag="e", name=f"e_{b}_{h}_{qs}_{kp}"
                    )
                    nc.scalar.activation(e[:, :, :], pss[:, :, :], Exp, scale=SCALE)
                    if kp == nk // 2 - 1:
                        # the final pair holds the causal diagonal
                        nc.vector.tensor_mul(
                            e[:, :, :], e[:, :, :], mask_sb[:, :, :]
                        )
                    for j in range(2):
                        ki = 2 * kp + j
                        nc.tensor.matmul(
                            ps_out,
                            v_tiles[b][:, ki, h * HD : (h + 1) * HD],
                            e[:, j, :],
                            start=(ki == 0),
                            stop=(ki == nk - 1),
                        )
                    ve = veng or nc.vector
                    if kp == 0:
                        ve.tensor_scalar_mul(acc[:, :, :], e[:, :, :], 1.0)
                    else:
                        ve.tensor_add(acc[:, :, :], acc[:, :, :], e[:, :, :])
                    if fill:
                        pf = ps_qk.tile(
                            [128, NSL], F32, tag="qk",
                            name=f"afill_{b}_{h}_{qs}_{kp}",
                        )
                        nc.tensor.matmul(
                            pf, scratch_bf[:, :128], scratch_bf,
                            start=True, stop=True,
                        )
                    if vjobs:
                        for _ in range(min(5, len(vjobs))):
                            vjobs.pop(0)()
                accf = acc_pool.tile(
                    [128, QW], F32R, tag="accf", name=f"accf_{b}_{h}_{qs}"
                )
                (veng or nc.vector).tensor_add(accf, acc[:, 0, :], acc[:, 1, :])
                return ps_out, accf

            def attn_norm(b, h, qs, ps_out, acc):
                """partition-reduce the folded denominator, normalize, store."""
                qsl = slice(qs * QW, (qs + 1) * QW)
                ps_sm = ps_s.tile([128, QW], F32, tag="sc", name=f"pm_{b}_{h}_{qs}")
                nc.tensor.matmul(ps_sm, ones_sb, acc, start=True, stop=True)
                rc = rcp_pool.tile([128, QW], F32, tag="rc", name=f"rc_{b}_{h}_{qs}")
                nc.vector.reciprocal_approx_fast(out=rc, in_=ps_sm)
                o = out_pool.tile([128, QW], BF16, tag="o", name=f"o_{b}_{h}_{qs}")
                nc.vector.tensor_mul(o, ps_out, rc)
                nc.sync.dma_start(out=out_ap[h, :, b, qsl], in_=o)

            def attn_scores_deep(b, h, qs, vjobs=None):
                """end-of-kernel variant: all score pairs are emitted ahead
                (rotating through 4 PSUM banks, borrowing the idle qk banks)
                so the exp stream never stalls the PE queue; the AV matmuls
                then stream densely."""
                nk = 2 * (qs + 1)
                qT = qk_tiles[(b, h)]
                kT = qk_tiles[(b, 2 + h)]
                qsl = slice(qs * QW, (qs + 1) * QW)
                es = []
                for kp in range(nk // 2):
                    pool, tg = (ps_s, "sc") if kp % 2 == 0 else (ps_qk, "qk")
                    pss = pool.tile(
                        [128, 2, QW], F32, tag=tg, name=f"scd_{b}_{h}_{qs}_{kp}"
                    )
                    for j in range(2):
                        ki = 2 * kp + j
                        nc.tensor.matmul(
                            pss[:, j, :],
                            kT[:, ki * 128 : (ki + 1) * 128],
                            qT[:, qsl],
                            start=(j == 0),
                            stop=(j == 1),
                        )
                    e = exp_pool.tile(
                        [128, 2, QW], F32R, tag="e", name=f"ed_{b}_{h}_{qs}_{kp}"
                    )
                    nc.scalar.activation(e[:, :, :], pss[:, :, :], Exp, scale=SCALE)
                    if kp == nk // 2 - 1:
                        nc.vector.tensor_mul(
                            e[:, :, :], e[:, :, :], mask_sb[:, :, :]
                        )
                    es.append(e)
                ps_out = ps_o.tile([128, QW], F32, tag="po", name=f"pod_{b}_{h}_{qs}")
                acc = acc_pool.tile(
                    [128, 2, QW], F32R, tag="acc", name=f"accd_{b}_{h}_{qs}"
                )
                for kp in range(nk // 2):
                    e = es[kp]
                    for j in range(2):
                        ki = 2 * kp + j
                        nc.tensor.matmul(
                            ps_out,
                            v_tiles[b][:, ki, h * HD : (h + 1) * HD],
                            e[:, j, :],
                            start=(ki == 0),
                            stop=(ki == nk - 1),
                        )
                    if kp == 0:
                        nc.vector.tensor_scalar_mul(acc[:, :, :], e[:, :, :], 1.0)
                    else:
                        nc.vector.tensor_add(acc[:, :, :], acc[:, :, :], e[:, :, :])
                    if vjobs:
                        for _ in range(min(5, len(vjobs))):
                            vjobs.pop(0)()
                accf = acc_pool.tile(
                    [128, QW], F32R, tag="accf", name=f"accfd_{b}_{h}_{qs}"
                )
                nc.vector.tensor_add(accf, acc[:, 0, :], acc[:, 1, :])
                return ps_out, accf

            def attn_slot(pairs, fill=False, vjobs=None, veng=None):
                """run a slot of (b, h, qs) chunks: all score loops first,
                then the normalize tails, so the denominator matmul never
                waits at the head of the PE queue. defer=True returns the
                held chunks so the caller can emit the norms after more PE
                work has been queued."""
                held = []
                for b_, h_, qs_ in pairs:
                    held.append(
                        (b_, h_, qs_)
                        + attn_scores(b_, h_, qs_, fill, vjobs, veng)
                    )
                for b_, h_, qs_, po_, acc_ in held:
                    attn_norm(b_, h_, qs_, po_, acc_)

            # b0's second half rides the sync queue in parallel with the
            # first half's gpsimd stream, so the projection never starves
            # during the cold-start window
            xs_b0h1 = issue_x(0, 1, eng=nc.sync)

            for b in range(B):
                for m in range(4):
                    qk_tiles[(b, m)] = qk_pool.tile(
                        [128, S], F32R, tag="qkt", name=f"qkt_{b}_{m}"
                    )
                v_tiles[b] = v_pool.tile(
                    [128, S // 128, 2 * HD], F32R, tag="vt", name=f"v_{b}"
                )

                last = b == B - 1
                for half in range(2):
                    if b == 0 and half == 1:
                        xs = xs_b0h1
                    else:
                        xs = issue_x(b, half)
                    pqs = proj_sweep(b, half, xs, 0)
                    # pipelined attention fills the PE while ACT/DVE run RoPE
                    if half == 0:
                        if b > 0:
                            attn_slot([(b - 1, h, 2) for h in range(HPC)])
                    else:
                        attn_slot([(b, h, 0) for h in range(HPC)])
                    rope_pair(b, half, 0, pqs)
                    v_pass(b, half, xs, 0)
                    pqs = proj_sweep(b, half, xs, 1)
                    if half == 0:
                        if b > 0:
                            attn_slot([(b - 1, h, 3) for h in range(HPC)])
                    else:
                        attn_slot([(b, h, 1) for h in range(HPC)])
                    rope_pair(b, half, 1, pqs)
                    if not (last and half == 1):
                        v_pass(b, half, xs, 1)

            # --- last-batch tail: only the diagonal remainders are left;
            # the final v projection interleaves as useful PE filler ---
            bL = B - 1
            vjobs = []
            for i in range(2):
                t = 2 + i
                pv = ps_v.tile(
                    [128, 2 * HD], F32, tag="v", name=f"pvL_{i}"
                )
                for kc in range(KC):
                    vjobs.append(
                        lambda pv=pv, kc=kc, t=t: nc.tensor.matmul(
                            pv,
                            xs[kc][:, t * 128 : (t + 1) * 128],
                            w_sb[kc][:, 4 * 128 : 6 * 128],
                            start=(kc == 0),
                            stop=(kc == KC - 1),
                        )
                    )
                vjobs.append(
                    lambda pv=pv, t=t: nc.vector.tensor_add(
                        v_tiles[bL][:, 4 + t, :], pv, bv_sb
                    )
                )
            held = []
            # give the PE immediate work while the first scores wait on
            # the RoPE chain of the last half
            for _ in range(8):
                vjobs.pop(0)()
            for h in range(HPC):
                held.append((bL, h, 2) + attn_scores_deep(bL, h, 2, vjobs))
            for b_, h_, qs_, po_, acc_ in held:
                attn_norm(b_, h_, qs_, po_, acc_)
            while vjobs:
                vjobs.pop(0)()
            # final slot: interleave both heads' score pairs so the exp
            # stream never breaks at the head boundary, then stream the AVs
            qs = 3
            nkL = 2 * (qs + 1)
            qslL = slice(qs * QW, (qs + 1) * QW)
            es = {}
            for kp in range(nkL // 2):
                for h in range(HPC):
                    pool, tg = (ps_s, "sc") if (2 * kp + h) % 2 == 0 else (
                        ps_qk, "qk"
                    )
                    pss = pool.tile(
                        [128, 2, QW], F32, tag=tg, name=f"scf_{h}_{kp}"
                    )
                    kT = qk_tiles[(bL, 2 + h)]
                    qT = qk_tiles[(bL, h)]
                    for j in range(2):
                        ki = 2 * kp + j
                        nc.tensor.matmul(
                            pss[:, j, :],
                            kT[:, ki * 128 : (ki + 1) * 128],
                            qT[:, qslL],
                            start=(j == 0),
                            stop=(j == 1),
                        )
                    e = exp_pool.tile(
                        [128, 2, QW], F32R, tag="e", name=f"ef_{h}_{kp}"
                    )
                    nc.scalar.activation(
                        e[:, :, :], pss[:, :, :], Exp, scale=SCALE
                    )
                    if kp == nkL // 2 - 1:
                        nc.vector.tensor_mul(
                            e[:, :, :], e[:, :, :], mask_sb[:, :, :]
                        )
                    es[(h, kp)] = e
            held = []
            for h in range(HPC):
                ps_out = ps_o.tile([128, QW], F32, tag="po", name=f"pof_{h}")
                acc = acc_pool.tile(
                    [128, 2, QW], F32R, tag="acc", name=f"accft_{h}"
                )
                for kp in range(nkL // 2):
                    e = es[(h, kp)]
                    for j in range(2):
                        ki = 2 * kp + j
                        nc.tensor.matmul(
                            ps_out,
                            v_tiles[bL][:, ki, h * HD : (h + 1) * HD],
                            e[:, j, :],
                            start=(ki == 0),
                            stop=(ki == nkL - 1),
                        )
                    if kp == 0:
                        nc.vector.tensor_scalar_mul(
                            acc[:, :, :], e[:, :, :], 1.0
                        )
                    else:
                        nc.vector.tensor_add(
                            acc[:, :, :], acc[:, :, :], e[:, :, :]
                        )
                accf = acc_pool.tile(
                    [128, QW], F32R, tag="accf", name=f"accff_{h}"
                )
                nc.vector.tensor_add(accf, acc[:, 0, :], acc[:, 1, :])
                held.append((bL, h, qs, ps_out, accf))
            for b_, h_, qs_, po_, acc_ in held:
                attn_norm(b_, h_, qs_, po_, acc_)

    nc.compile()
    return nc


def _prep_shared(hidden_states):
    x2 = np.ascontiguousarray(hidden_states.reshape(T, D).T)  # [D, T]
    x_host = np.ascontiguousarray(
        x2.reshape(KC, 128, T).transpose(1, 0, 2)
    )  # [128, KC, T]

    inv = 1.0 / (ROPE_BASE ** (np.arange(0, HD, 2, dtype=np.float64) / HD))
    f = np.outer(inv, np.arange(S, dtype=np.float64))  # [64, S]
    cosT = np.concatenate([np.cos(f), np.cos(f)], axis=0).astype(np.float32)
    sinS = np.concatenate([np.sin(f), np.sin(f)], axis=0).astype(np.float32)

    p = np.arange(128)[:, None]
    fcol = np.arange(QW)[None, :]
    masks = np.stack(
        [(fcol >= p + o).astype(np.float32) for o in (0, 128)], axis=1
    )  # [128, 2, QW]
    masks = np.ascontiguousarray(masks)
    # rotate_half is done on-chip as a partition rotation by 64 (a pure
    # permutation); the sign of rotate_half lives in the sin table:
    # rows 0..63 carry -sin, rows 64..127 carry +sin.
    sinS[:64] *= -1.0
    return x_host, cosT, sinS, masks


def _core_rows(c):
    h0, h1 = 2 * c, 2 * c + 1
    rows = []
    for part in range(3):  # q, k, v blocks
        for h in (h0, h1):
            base = h * 3 * HD + part * HD
            rows.extend(range(base, base + HD))
    return np.asarray(rows)


def _prep_core(w_qkv, b_qkv, c):
    rows = _core_rows(c)
    wT = np.ascontiguousarray(w_qkv[rows, :].T)  # [D, 768]
    w_host = np.ascontiguousarray(
        wT.reshape(KC, 128, 6 * HD).transpose(1, 0, 2)
    )  # [128, KC, 768]
    b_sel = b_qkv[rows]
    bqk = np.ascontiguousarray(b_sel[: 4 * 128].reshape(4, 128).T)  # [128, 4]
    bv = np.ascontiguousarray(
        np.broadcast_to(b_sel[4 * 128 :], (128, 2 * HD))
    )  # [128, 256]
    return w_host, bqk, bv


def _make_in_maps(hidden_states, w_qkv, b_qkv):
    x_host, cosT, sinS, masks = _prep_shared(hidden_states)
    in_maps = []
    for c in range(NCORES):
        w_host, bqk, bv = _prep_core(w_qkv, b_qkv, c)
        in_maps.append(
            {
                "x": x_host.astype(ml_dtypes.bfloat16),
                "w": w_host.astype(ml_dtypes.bfloat16),
                "bqk": bqk,
                "bv": bv,
                "cosT": cosT.astype(ml_dtypes.bfloat16),
                "sinS": sinS.astype(ml_dtypes.bfloat16),
                "masks": masks.astype(ml_dtypes.bfloat16),
                "ones": np.ones((128, 128), np.float32),
            }
        )
    return in_maps


def _assemble(results):
    outs = np.stack(
        [results[c]["out"].astype(np.float32) for c in range(NCORES)]
    )
    # [NCORES, HPC, HD, B, S] -> [B, S, H*HD]
    return np.ascontiguousarray(
        outs.reshape(H, HD, B, S).transpose(2, 3, 0, 1).reshape(B, S, D)
    )


def run(hidden_states, w_qkv, b_qkv, trace=False):
    from concourse.bass_utils import run_bass_kernel_spmd

    if "nc" not in _CACHE:
        _CACHE["nc"] = _build_program()
    nc = _CACHE["nc"]
    in_maps = _make_in_maps(
        np.asarray(hidden_states, dtype=np.float32),
        np.asarray(w_qkv, dtype=np.float32),
        np.asarray(b_qkv, dtype=np.float32),
    )
    res = run_bass_kernel_spmd(
        nc, in_maps, core_ids=list(range(NCORES)), trace=trace
    )
    out = _assemble(res.results)
    return out, res


def kernel(hidden_states, w_qkv, b_qkv):
    trace = os.environ.get("KERNEL_TRACE", "0") == "1"
    out, _res = run(hidden_states, w_qkv, b_qkv, trace=trace)
    return out


# revision 42
# speedup vs baseline: 1.0121x; 1.0121x over previous
"""GPT-NeoX attention (B=4, S=1024, D=2048, H=16) on 8 TRN2 NeuronCores.

Tensor-parallel over heads: 2 heads per core. Each core computes its slice
of the fused QKV projection, RoPE, causal attention, and writes the
transposed per-head output [hd, S]; the host concatenates heads.

All on-chip matmuls use float32r (full PE speed at free-dim>=256) with
fp32 PSUM accumulation. Layouts avoid on-chip transposes:
  - x is fed transposed  xT[feature, token]
  - q,k are produced transposed  qT/kT[hd, token]  (RoPE applied in place)
  - v is produced natural  v[token, hd]  via x-stationary matmuls
  - scores are computed transposed  sT[k_token, q_token]
  - out is produced transposed  oT[hd, q_token] = v.T @ expT
  - softmax sum over k = ones-matmul; normalization applied to oT

Scheduling structure (vs the naive version):
  - x and w stream per-kc chunk on separate DMA queues so the first
    matmul waits for ~320KB, not the whole 10.5MB prefetch.
  - The projection loop is kc-outer with 4 open PSUM groups, so compute
    tracks chunk arrival during the DMA-paced first batch.
  - Attention runs on 256-wide q-chunks (finer causal trim) and is
    software-pipelined into the RoPE/bias windows of the next half.
"""

import os

import numpy as np

import ml_dtypes

import concourse.bass as bass
import concourse.tile as tile
from concourse import bacc, mybir

# Problem constants (contract: nn_GPTNeoXAttention, fixed shapes)
B, S, D = 4, 1024, 2048
H = 16
HD = 128  # head dim
NCORES = 8
HPC = H // NCORES  # heads per core
ROPE_BASE = 10000.0
T = B * S  # 4096 tokens
KC = D // 128  # 16 contraction chunks of the model dim
NSL = 512  # token-slice width (half a sequence)
QW = 256  # attention q-chunk width
SCALE = 1.0 / float(np.sqrt(HD))

F32 = mybir.dt.float32
F32R = mybir.dt.float32r
BF16 = mybir.dt.bfloat16

_CACHE = {}


def _build_program():
    nc = bacc.Bacc(
        "TRN2", target_bir_lowering=False, debug=False, num_devices=NCORES
    )

    x_d = nc.dram_tensor("x", [128, KC, T], BF16, kind="ExternalInput")
    w_d = nc.dram_tensor("w", [128, KC, 6 * HD], BF16, kind="ExternalInput")
    bqk_d = nc.dram_tensor("bqk", [128, 4], F32, kind="ExternalInput")
    bv_d = nc.dram_tensor("bv", [128, 2 * HD], F32, kind="ExternalInput")
    cos_d = nc.dram_tensor("cosT", [128, S], BF16, kind="ExternalInput")
    sin_d = nc.dram_tensor("sinS", [128, S], BF16, kind="ExternalInput")
    mask_d = nc.dram_tensor("masks", [128, 2, QW], BF16, kind="ExternalInput")
    ones_d = nc.dram_tensor("ones", [128, 128], F32R, kind="ExternalInput")
    out_d = nc.dram_tensor("out", [HPC, HD, B, S], BF16, kind="ExternalOutput")

    x_ap = x_d.ap()
    w_ap = w_d.ap()
    out_ap = out_d.ap()

    Exp = mybir.ActivationFunctionType.Exp
    Identity = mybir.ActivationFunctionType.Identity

    with tile.TileContext(nc) as tc:
        with (
            tc.tile_pool(name="singles", bufs=1) as singles,
            tc.tile_pool(name="xin", bufs=2) as xin_pool,
            tc.tile_pool(name="qk", bufs=8) as qk_pool,
            tc.tile_pool(name="vp", bufs=2) as v_pool,
            tc.tile_pool(name="qbp", bufs=3) as qb_pool,
            tc.tile_pool(name="t2p", bufs=2) as t2_pool,
            tc.tile_pool(name="expp", bufs=8) as exp_pool,
            tc.tile_pool(name="outp", bufs=4) as out_pool,
            tc.tile_pool(name="rcp", bufs=3) as rcp_pool,
            tc.tile_pool(name="accp", bufs=3) as acc_pool,
            tc.tile_pool(name="ps_qk", bufs=2, space="PSUM") as ps_qk,
            tc.tile_pool(name="ps_v", bufs=2, space="PSUM") as ps_v,
            tc.tile_pool(name="ps_s", bufs=2, space="PSUM") as ps_s,
            tc.tile_pool(name="ps_o", bufs=2, space="PSUM") as ps_o,
        ):
            # --- HAM warmup: dummy matmuls run during the DMA-bound
            # prefetch so the PE clock-gate opens before real compute ---
            scratch = singles.tile([128, NSL], F32, tag="scratch")
            nc.gpsimd.memset(scratch[:], 0.0)
            scratch_bf = singles.tile([128, NSL], BF16, tag="scratchb")
            nc.gpsimd.memset(scratch_bf[:], 0.0)
            for wi in range(6):
                pw = ps_qk.tile([128, NSL], F32, tag="qk", name=f"warm_{wi}")
                nc.tensor.matmul(
                    pw, scratch[:, :128], scratch, start=True, stop=True
                )

            # --- weights stream per-kc, split by use: the q-head columns
            # arrive first so the first projection sweep is paced by ~1MB,
            # then k columns; v columns ride the sync queue after singles ---
            w_sb = []
            for kc in range(KC):
                wt = singles.tile(
                    [128, 6 * HD], BF16, tag=f"w{kc}", name=f"w_sb_{kc}"
                )
                w_sb.append(wt)
            for kc in range(KC):
                nc.scalar.dma_start(
                    out=w_sb[kc][:, 0:256], in_=w_ap[:, kc, 0:256]
                )
            for kc in range(KC):
                nc.scalar.dma_start(
                    out=w_sb[kc][:, 256:512], in_=w_ap[:, kc, 256:512]
                )
            # --- small constants on the sync queue (idle at start) ---
            bqk_sb = singles.tile([128, 4], F32, tag="bqk")
            nc.sync.dma_start(out=bqk_sb, in_=bqk_d.ap())
            cos_sb = singles.tile([128, S], BF16, tag="cos")
            nc.sync.dma_start(out=cos_sb, in_=cos_d.ap())
            sin_sb = singles.tile([128, S], BF16, tag="sin")
            nc.sync.dma_start(out=sin_sb, in_=sin_d.ap())
            bv_sb = singles.tile([128, 2 * HD], F32, tag="bv")
            nc.sync.dma_start(out=bv_sb, in_=bv_d.ap())
            mask_sb = singles.tile([128, 2, QW], BF16, tag="mask")
            nc.sync.dma_start(out=mask_sb, in_=mask_d.ap())
            # ones[128,128] lhsT: ones.T @ expT = sum over k on all partitions
            ones_sb = singles.tile([128, 128], F32R, tag="ones")
            nc.sync.dma_start(out=ones_sb, in_=ones_d.ap())
            for kc in range(KC):
                nc.sync.dma_start(
                    out=w_sb[kc][:, 512:768], in_=w_ap[:, kc, 512:768]
                )

            qk_tiles = {}  # (b, m) -> feature-major q/k tile [128, S]
            v_tiles = {}  # b -> natural v tile [128, 8, 2*HD]

            def issue_x(b, half, eng=None):
                t0 = b * S + half * NSL
                xs = []
                for kc in range(KC):
                    xt = xin_pool.tile(
                        [128, NSL], BF16, tag=f"x{kc}", name=f"x_{b}_{half}_{kc}"
                    )
                    (eng or nc.gpsimd).dma_start(
                        out=xt, in_=x_ap[:, kc, t0 : t0 + NSL]
                    )
                    xs.append(xt)
                return xs

            def proj_sweep(b, half, xs, sweep):
                """q heads (sweep=0) or k heads (sweep=1): two qk PSUM
                groups accumulated kc-outer so compute tracks DMA arrival."""
                pqs = [
                    ps_qk.tile(
                        [128, NSL], F32, tag="qk", name=f"pq_{b}_{half}_{sweep}_{i}"
                    )
                    for i in range(2)
                ]
                for kc in range(KC):
                    for i in range(2):
                        m = 2 * sweep + i
                        nc.tensor.matmul(
                            pqs[i],
                            w_sb[kc][:, m * 128 : (m + 1) * 128],
                            xs[kc],
                            start=(kc == 0),
                            stop=(kc == KC - 1),
                        )
                    nfill = 1 if (b == 0 and half == 0 and kc < KC - 1) else 0
                    for fi in range(nfill):
                        # keep the PE clock-gate open while DMA paces b0:
                        # dependency-free filler matmuls bridge the gap
                        pf = ps_s.tile(
                            [128, 2, QW], F32, tag="sc",
                            name=f"fill_{b}_{half}_{sweep}_{kc}_{fi}",
                        )
                        nc.tensor.matmul(
                            pf[:, :, :], scratch_bf[:, :128], scratch_bf,
                            start=True, stop=True,
                        )
                return pqs

            def v_pass(b, half, xs, pair):
                """natural-layout v for token tiles (2*pair, 2*pair+1)."""
                for i in range(2):
                    t = 2 * pair + i
                    pv = ps_v.tile(
                        [128, 2 * HD], F32, tag="v", name=f"pv_{b}_{half}_{t}"
                    )
                    for kc in range(KC):
                        nc.tensor.matmul(
                            pv,
                            xs[kc][:, t * 128 : (t + 1) * 128],
                            w_sb[kc][:, 4 * 128 : 6 * 128],
                            start=(kc == 0),
                            stop=(kc == KC - 1),
                        )
                    nc.vector.tensor_add(
                        v_tiles[b][:, half * 4 + t, :], pv, bv_sb
                    )

            def rope_pair(b, half, sweep, pqs):
                """bias + RoPE for the two feature blocks of one sweep.
                rotate_half = partition rotation by 64 via SBUF-to-SBUF DMA
                (sign folded into the sin table)."""
                sl = slice(half * NSL, (half + 1) * NSL)
                for i in range(2):
                    m = 2 * sweep + i
                    qb = qb_pool.tile(
                        [128, NSL], F32R, tag="qb", name=f"qb_{b}_{half}_{m}"
                    )
                    nc.scalar.activation(
                        qb, pqs[i], Identity, bias=bqk_sb[:, m : m + 1], scale=1.0
                    )
                    qrot = t2_pool.tile(
                        [128, NSL], F32R, tag="t2", name=f"t2_{b}_{half}_{m}"
                    )
                    nc.sync.dma_start(out=qrot[0:64, :], in_=qb[64:128, :])
                    nc.sync.dma_start(out=qrot[64:128, :], in_=qb[0:64, :])
                    dst = qk_tiles[(b, m)][:, sl]
                    nc.vector.tensor_mul(qrot, qrot, sin_sb[:, sl])
                    nc.vector.tensor_mul(dst, qb, cos_sb[:, sl])
                    nc.vector.tensor_add(dst, dst, qrot)

            def attn_scores(b, h, qs, fill=False, vjobs=None, veng=None):
                """scores/exp/mask/AV for one 256-wide q-chunk. Score chunks
                are paired into one PSUM bank (one accumulation group writing
                disjoint halves) so exp and the DVE denominator accumulation
                run on [128,512] tiles. fill=True inserts dependency-free
                filler matmuls so the PE clock-gate stays open while the
                chunk chain waits on ACT/DVE."""
                nk = 2 * (qs + 1)
                qT = qk_tiles[(b, h)]
                kT = qk_tiles[(b, 2 + h)]
                qsl = slice(qs * QW, (qs + 1) * QW)
                ps_out = ps_o.tile([128, QW], F32, tag="po", name=f"po_{b}_{h}_{qs}")
                acc = acc_pool.tile(
                    [128, 2, QW], F32R, tag="acc", name=f"acc_{b}_{h}_{qs}"
                )
                for kp in range(nk // 2):
                    pss = ps_s.tile(
                        [128, 2, QW], F32, tag="sc", name=f"sc_{b}_{h}_{qs}_{kp}"
                    )
                    for j in range(2):
                        ki = 2 * kp + j
                        nc.tensor.matmul(
                            pss[:, j, :],
                            kT[:, ki * 128 : (ki + 1) * 128],
                            qT[:, qsl],
                            start=(j == 0),
                            stop=(j == 1),
                        )
                    e = exp_pool.tile(
                        [128, 2, QW], F32R, tag="e", name=f"e_{b}_{h}_{qs}_{kp}"
                    )
                    nc.scalar.activation(e[:, :, :], pss[:, :, :], Exp, scale=SCALE)
                    if kp == nk // 2 - 1:
                        # the final pair holds the causal diagonal
                        nc.vector.tensor_mul(
                            e[:, :, :], e[:, :, :], mask_sb[:, :, :]
                        )
                    for j in range(2):
                        ki = 2 * kp + j
                        nc.tensor.matmul(
                            ps_out,
                            v_tiles[b][:, ki, h * HD : (h + 1) * HD],
                            e[:, j, :],
                            start=(ki == 0),
                            stop=(ki == nk - 1),
                        )
                    ve = veng or nc.vector
                    if kp == 0:
                        ve.tensor_scalar_mul(acc[:, :, :], e[:, :, :], 1.0)
                    else:
                        ve.tensor_add(acc[:, :, :], acc[:, :, :], e[:, :, :])
                    if fill:
                        pf = ps_qk.tile(
                            [128, NSL], F32, tag="qk",
                            name=f"afill_{b}_{h}_{qs}_{kp}",
                        )
                        nc.tensor.matmul(
                            pf, scratch_bf[:, :128], scratch_bf,
                            start=True, stop=True,
                        )
                    if vjobs:
                        for _ in range(min(5, len(vjobs))):
                            vjobs.pop(0)()
                accf = acc_pool.tile(
                    [128, QW], F32R, tag="accf", name=f"accf_{b}_{h}_{qs}"
                )
                (veng or nc.vector).tensor_add(accf, acc[:, 0, :], acc[:, 1, :])
                return ps_out, accf

            def attn_norm(b, h, qs, ps_out, acc):
                """partition-reduce the folded denominator, normalize, store."""
                qsl = slice(qs * QW, (qs + 1) * QW)
                ps_sm = ps_s.tile([128, QW], F32, tag="sc", name=f"pm_{b}_{h}_{qs}")
                nc.tensor.matmul(ps_sm, ones_sb, acc, start=True, stop=True)
                rc = rcp_pool.tile([128, QW], F32, tag="rc", name=f"rc_{b}_{h}_{qs}")
                nc.vector.reciprocal_approx_fast(out=rc, in_=ps_sm)
                o = out_pool.tile([128, QW], BF16, tag="o", name=f"o_{b}_{h}_{qs}")
                nc.vector.tensor_mul(o, ps_out, rc)
                nc.sync.dma_start(out=out_ap[h, :, b, qsl], in_=o)

            def attn_scores_deep(b, h, qs, vjobs=None):
                """end-of-kernel variant: all score pairs are emitted ahead
                (rotating through 4 PSUM banks, borrowing the idle qk banks)
                so the exp stream never stalls the PE queue; the AV matmuls
                then stream densely."""
                nk = 2 * (qs + 1)
                qT = qk_tiles[(b, h)]
                kT = qk_tiles[(b, 2 + h)]
                qsl = slice(qs * QW, (qs + 1) * QW)
                es = []
                for kp in range(nk // 2):
                    pool, tg = (ps_s, "sc") if kp % 2 == 0 else (ps_qk, "qk")
                    pss = pool.tile(
                        [128, 2, QW], F32, tag=tg, name=f"scd_{b}_{h}_{qs}_{kp}"
                    )
                    for j in range(2):
                        ki = 2 * kp + j
                        nc.tensor.matmul(
                            pss[:, j, :],
                            kT[:, ki * 128 : (ki + 1) * 128],
                            qT[:, qsl],
                            start=(j == 0),
                            stop=(j == 1),
                        )
                    e = exp_pool.tile(
                        [128, 2, QW], F32R, tag="e", name=f"ed_{b}_{h}_{qs}_{kp}"
                    )
                    nc.scalar.activation(e[:, :, :], pss[:, :, :], Exp, scale=SCALE)
                    if kp == nk // 2 - 1:
                        nc.vector.tensor_mul(
                            e[:, :, :], e[:, :, :], mask_sb[:, :, :]
                        )
                    es.append(e)
                ps_out = ps_o.tile([128, QW], F32, tag="po", name=f"pod_{b}_{h}_{qs}")
                acc = acc_pool.tile(
                    [128, 2, QW], F32R, tag="acc", name=f"accd_{b}_{h}_{qs}"
                )
                for kp in range(nk // 2):
                    e = es[kp]
                    for j in range(2):
                        ki = 2 * kp + j
                        nc.tensor.matmul(
                            ps_out,
                            v_tiles[b][:, ki, h * HD : (h + 1) * HD],
                            e[:, j, :],
                            start=(ki == 0),
                            stop=(ki == nk - 1),
                        )
                    if kp == 0:
                        nc.vector.tensor_scalar_mul(acc[:, :, :], e[:, :, :], 1.0)
                    else:
                        nc.vector.tensor_add(acc[:, :, :], acc[:, :, :], e[:, :, :])
                    if vjobs:
                        for _ in range(min(5, len(vjobs))):
                            vjobs.pop(0)()
                accf = acc_pool.tile(
                    [128, QW], F32R, tag="accf", name=f"accfd_{b}_{h}_{qs}"
                )
                nc.vector.tensor_add(accf, acc[:, 0, :], acc[:, 1, :])
                return ps_out, accf

            def attn_slot(pairs, fill=False, vjobs=None, veng=None):
                """run a slot of (b, h, qs) chunks: all score loops first,
                then the normalize tails, so the denominator matmul never
                waits at the head of the PE queue. defer=True returns the
                held chunks so the caller can emit the norms after more PE
                work has been queued."""
                held = []
                for b_, h_, qs_ in pairs:
                    held.append(
                        (b_, h_, qs_)
                        + attn_scores(b_, h_, qs_, fill, vjobs, veng)
                    )
                for b_, h_, qs_, po_, acc_ in held:
                    attn_norm(b_, h_, qs_, po_, acc_)

            # b0's second half rides the sync queue in parallel with the
            # first half's gpsimd stream, so the projection never starves
            # during the cold-start window
            xs_b0h1 = issue_x(0, 1, eng=nc.sync)

            for b in range(B):
                for m in range(4):
                    qk_tiles[(b, m)] = qk_pool.tile(
                        [128, S], F32R, tag="qkt", name=f"qkt_{b}_{m}"
                    )
                v_tiles[b] = v_pool.tile(
                    [128, S // 128, 2 * HD], F32R, tag="vt", name=f"v_{b}"
                )

                last = b == B - 1
                for half in range(2):
                    if b == 0 and half == 1:
                        xs = xs_b0h1
                    else:
                        xs = issue_x(b, half)
                    pqs = proj_sweep(b, half, xs, 0)
                    # pipelined attention fills the PE while ACT/DVE run RoPE
                    if half == 0:
                        if b > 0:
                            attn_slot([(b - 1, h, 2) for h in range(HPC)])
                    else:
                        attn_slot([(b, h, 0) for h in range(HPC)])
                    rope_pair(b, half, 0, pqs)
                    v_pass(b, half, xs, 0)
                    pqs = proj_sweep(b, half, xs, 1)
                    if half == 0:
                        if b > 0:
                            attn_slot([(b - 1, h, 3) for h in range(HPC)])
                    else:
                        attn_slot([(b, h, 1) for h in range(HPC)])
                    rope_pair(b, half, 1, pqs)
                    if not (last and half == 1):
                        v_pass(b, half, xs, 1)

            # --- last-batch tail: only the diagonal remainders are left;
            # the final v projection interleaves as useful PE filler ---
            bL = B - 1
            vjobs = []
            for i in range(2):
                t = 2 + i
                pv = ps_v.tile(
                    [128, 2 * HD], F32, tag="v", name=f"pvL_{i}"
                )
                for kc in range(KC):
                    vjobs.append(
                        lambda pv=pv, kc=kc, t=t: nc.tensor.matmul(
                            pv,
                            xs[kc][:, t * 128 : (t + 1) * 128],
                            w_sb[kc][:, 4 * 128 : 6 * 128],
                            start=(kc == 0),
                            stop=(kc == KC - 1),
                        )
                    )
                vjobs.append(
                    lambda pv=pv, t=t: nc.vector.tensor_add(
                        v_tiles[bL][:, 4 + t, :], pv, bv_sb
                    )
                )
            held = []
            # give the PE immediate work while the first scores wait on
            # the RoPE chain of the last half
            for _ in range(8):
                vjobs.pop(0)()
            for h in range(HPC):
                held.append((bL, h, 2) + attn_scores_deep(bL, h, 2, vjobs))
            for b_, h_, qs_, po_, acc_ in held:
                attn_norm(b_, h_, qs_, po_, acc_)
            while vjobs:
                vjobs.pop(0)()
            # final slot: interleave both heads' score pairs so the exp
            # stream never breaks at the head boundary, then stream the AVs
            qs = 3
            nkL = 2 * (qs + 1)
            qslL = slice(qs * QW, (qs + 1) * QW)
            es = {}
            for kp in range(nkL // 2):
                for h in range(HPC):
                    pool, tg = (ps_s, "sc") if (2 * kp + h) % 2 == 0 else (
                        ps_qk, "qk"
                    )
                    pss = pool.tile(
                        [128, 2, QW], F32, tag=tg, name=f"scf_{h}_{kp}"
                    )
                    kT = qk_tiles[(bL, 2 + h)]
                    qT = qk_tiles[(bL, h)]
                    for j in range(2):
                        ki = 2 * kp + j
                        nc.tensor.matmul(
                            pss[:, j, :],
                            kT[:, ki * 128 : (ki + 1) * 128],
                            qT[:, qslL],
                            start=(j == 0),
                            stop=(j == 1),
                        )
                    e = exp_pool.tile(
                        [128, 2, QW], F32R, tag="e", name=f"ef_{h}_{kp}"
                    )
                    nc.scalar.activation(
                        e[:, :, :], pss[:, :, :], Exp, scale=SCALE
                    )
                    if kp == nkL // 2 - 1:
                        nc.vector.tensor_mul(
                            e[:, :, :], e[:, :, :], mask_sb[:, :, :]
                        )
                    es[(h, kp)] = e
            held = []
            for h in range(HPC):
                ps_out = ps_o.tile([128, QW], F32, tag="po", name=f"pof_{h}")
                acc = acc_pool.tile(
                    [128, 2, QW], F32R, tag="acc", name=f"accft_{h}"
                )
                for kp in range(nkL // 2):
                    e = es[(h, kp)]
                    for j in range(2):
                        ki = 2 * kp + j
                        nc.tensor.matmul(
                            ps_out,
                            v_tiles[bL][:, ki, h * HD : (h + 1) * HD],
                            e[:, j, :],
                            start=(ki == 0),
                            stop=(ki == nkL - 1),
                        )
                    if kp == 0:
                        nc.vector.tensor_scalar_mul(
                            acc[:, :, :], e[:, :, :], 1.0
                        )
                    else:
                        nc.vector.tensor_add(
                            acc[:, :, :], acc[:, :, :], e[:, :, :]
                        )
                accf = acc_pool.tile(
                    [128, QW], F32R, tag="accf", name=f"accff_{h}"
                )
                nc.vector.tensor_add(accf, acc[:, 0, :], acc[:, 1, :])
                held.append((bL, h, qs, ps_out, accf))
            for b_, h_, qs_, po_, acc_ in held:
                attn_norm(b_, h_, qs_, po_, acc_)

    nc.compile()
    return nc


def _prep_shared(hidden_states):
    x2 = np.ascontiguousarray(hidden_states.reshape(T, D).T)  # [D, T]
    x_host = np.ascontiguousarray(
        x2.reshape(KC, 128, T).transpose(1, 0, 2)
    )  # [128, KC, T]

    inv = 1.0 / (ROPE_BASE ** (np.arange(0, HD, 2, dtype=np.float64) / HD))
    f = np.outer(inv, np.arange(S, dtype=np.float64))  # [64, S]
    cosT = np.concatenate([np.cos(f), np.cos(f)], axis=0).astype(np.float32)
    sinS = np.concatenate([np.sin(f), np.sin(f)], axis=0).astype(np.float32)

    p = np.arange(128)[:, None]
    fcol = np.arange(QW)[None, :]
    masks = np.stack(
        [(fcol >= p + o).astype(np.float32) for o in (0, 128)], axis=1
    )  # [128, 2, QW]
    masks = np.ascontiguousarray(masks)
    # rotate_half is done on-chip as a partition rotation by 64 (a pure
    # permutation); the sign of rotate_half lives in the sin table:
    # rows 0..63 carry -sin, rows 64..127 carry +sin.
    sinS[:64] *= -1.0
    return x_host, cosT, sinS, masks


def _core_rows(c):
    h0, h1 = 2 * c, 2 * c + 1
    rows = []
    for part in range(3):  # q, k, v blocks
        for h in (h0, h1):
            base = h * 3 * HD + part * HD
            rows.extend(range(base, base + HD))
    return np.asarray(rows)


def _prep_core(w_qkv, b_qkv, c):
    rows = _core_rows(c)
    wT = np.ascontiguousarray(w_qkv[rows, :].T)  # [D, 768]
    w_host = np.ascontiguousarray(
        wT.reshape(KC, 128, 6 * HD).transpose(1, 0, 2)
    )  # [128, KC, 768]
    b_sel = b_qkv[rows]
    bqk = np.ascontiguousarray(b_sel[: 4 * 128].reshape(4, 128).T)  # [128, 4]
    bv = np.ascontiguousarray(
        np.broadcast_to(b_sel[4 * 128 :], (128, 2 * HD))
    )  # [128, 256]
    return w_host, bqk, bv


def _make_in_maps(hidden_states, w_qkv, b_qkv):
    x_host, cosT, sinS, masks = _prep_shared(hidden_states)
    in_maps = []
    for c in range(NCORES):
        w_host, bqk, bv = _prep_core(w_qkv, b_qkv, c)
        in_maps.append(
            {
                "x": x_host.astype(ml_dtypes.bfloat16),
                "w": w_host.astype(ml_dtypes.bfloat16),
                "bqk": bqk,
                "bv": bv,
                "cosT": cosT.astype(ml_dtypes.bfloat16),
                "sinS": sinS.astype(ml_dtypes.bfloat16),
                "masks": masks.astype(ml_dtypes.bfloat16),
                "ones": np.ones((128, 128), np.float32),
            }
        )
    return in_maps


def _assemble(results):
    outs = np.stack(
        [results[c]["out"].astype(np.float32) for c in range(NCORES)]
    )
    # [NCORES, HPC, HD, B, S] -> [B, S, H*HD]
    return np.ascontiguousarray(
        outs.reshape(H, HD, B, S).transpose(2, 3, 0, 1).reshape(B, S, D)
    )


def run(hidden_states, w_qkv, b_qkv, trace=False):
    from concourse.bass_utils import run_bass_kernel_spmd

    if "nc" not in _CACHE:
        _CACHE["nc"] = _build_program()
    nc = _CACHE["nc"]
    in_maps = _make_in_maps(
        np.asarray(hidden_states, dtype=np.float32),
        np.asarray(w_qkv, dtype=np.float32),
        np.asarray(b_qkv, dtype=np.float32),
    )
    res = run_bass_kernel_spmd(
        nc, in_maps, core_ids=list(range(NCORES)), trace=trace
    )
    out = _assemble(res.results)
    return out, res


def kernel(hidden_states, w_qkv, b_qkv):
    trace = os.environ.get("KERNEL_TRACE", "0") == "1"
    out, _res = run(hidden_states, w_qkv, b_qkv, trace=trace)
    return out
